# revision 3
# baseline (speedup 1.0000x reference)
"""MAMConv1d Trainium2 kernel.

Y[b,o,l] = max_{c,k}(W[o,c,k] * x[b,c,l+k]) + min_{c,k}(...) + bias[o]
B=8, C=64, L=1024, O=64, K=3, stride=1, Lout=1022.

Strategy (8 NeuronCores, data-parallel over batch B). Per core b:
- Products are formed on the TensorEngine via block-diagonal matmuls:
  P[l, k*512 + c*8 + oi] = x[c, s+k+l] * W[o,c,k] with lhsT = x-window
  [64c, 128l] stationary and rhs = diag-expanded fp16 weights [64c, 512]
  (8 output channels per matmul). ScalarE casts each PSUM block to fp16;
  the VectorEngine runs the k-combine and a fully packed-2x c-halving
  tree (o innermost, so every level down to width 1 is a 2x
  tensor_tensor; no 1x tensor_reduce).
- The DVE is the saturated engine (~132us busy of ~140us): it must
  execute every max/min op. Measured constraints that pin this design:
  GpSimd rejects compare ALU ops (walrus ISA) and, worse, any running
  Pool-engine op stalls concurrent DVE ops even on disjoint tiles, so
  nothing (ysum included) runs there. fp16 PSUM matmul output is
  TRN3-only, so the ACT cast (~98us, hidden under DVE) is mandatory.
  tensor_reduce has no DVE perf modes (1x), hence the TT tree.
- bias is folded in on the host after the gather; output is written
  l-major [1024, 64] per core; host transposes/gathers.
"""

import numpy as np

_B, _C, _L = 8, 64, 1024
_O, _K = 64, 3
_LOUT = (_L - _K) + 1  # 1022
_LPAD = _L + 8
_OG = 8  # o-channels per matmul / reduce group
_NT = 8  # l-tiles of 128

_cache = {}

_MM_DTYPE = "float16"


def _build_module():
    import concourse.bacc as bacc
    import concourse.bass as bass
    import concourse.mybir as mybir
    import concourse.tile as tile

    f32 = mybir.dt.float32
    mmdt = getattr(mybir.dt, _MM_DTYPE)
    nc = bacc.Bacc("TRN2", target_bir_lowering=False, debug=False)

    x_d = nc.dram_tensor("x", [_C, _LPAD], mmdt, kind="ExternalInput")
    wd_d = nc.dram_tensor("wd", [_O // _OG, _C, _K * _OG * _C], mmdt, kind="ExternalInput")
    yt_d = nc.dram_tensor("yt", [_NT * 128, _O], f32, kind="ExternalOutput")

    n_og = _O // _OG  # 8 groups of 8 output channels
    gcols = _K * _OG * _C  # 1536 product columns per group

    with tile.TileContext(nc) as tc:
        with (
            tc.tile_pool(name="const", bufs=1) as cpool,
            tc.tile_pool(name="psum", bufs=2, space=bass.MemorySpace.PSUM) as ppool,
            tc.tile_pool(name="outp", bufs=3) as opool,
        ):
            gsz0 = _OG * _C  # 512
            xs = cpool.tile([_C, _LPAD], mmdt)
            wds = [cpool.tile([_C, gcols], mmdt, name=f"wds{og}") for og in range(n_og)]
            # Startup: first matmul needs xs[:, 0:130] + wds[0] k0; spread the
            # three wds[0] k-slabs across three DGE queues so they land in
            # parallel instead of serializing at ~600ns each.
            nc.scalar.dma_start(xs[:, 0:132], x_d[:, 0:132])
            nc.sync.dma_start(wds[0][:, 0:256], wd_d[0][:, 0:256])
            nc.sync.dma_start(wds[0][:, 256:512], wd_d[0][:, 256:512])
            nc.scalar.dma_start(wds[0][:, 512:1024], wd_d[0][:, 512:1024])
            nc.sync.dma_start(wds[0][:, 1024:1536], wd_d[0][:, 1024:1536])
            for xi, (a, b) in enumerate([(132, 432), (432, 732), (732, _LPAD)]):
                nc.scalar.dma_start(xs[:, a:b], x_d[:, a:b])
            for og in range(1, n_og):
                for k in range(_K):
                    cs = slice(k * gsz0, (k + 1) * gsz0)
                    nc.sync.dma_start(wds[og][:, cs], wd_d[og][:, cs])

            f16 = mybir.dt.float16
            gsz = _OG * _C  # 512 columns per k-plane

            mx, mn = mybir.AluOpType.max, mybir.AluOpType.min

            for t in range(_NT):
                s = 128 * t
                ymax = opool.tile([128, _O], f16, tag="ymax")
                ymin = opool.tile([128, _O], f16, tag="ymin")
                # graduated chunks fill the DVE pipeline early
                sched = {0: (1, 1, 2, 4)}.get(t, (8,))
                og_start = 0
                for _J in sched:
                    Sf = opool.tile([128, _K, 8, gsz], f16, tag="S", bufs=3)
                    S = Sf[:, :, :_J, :]
                    for j in range(_J):
                        og = og_start + j
                        P = ppool.tile([128, gcols], f32, tag="P")
                        for k in range(_K):
                            nc.tensor.matmul(
                                P[:, k * gsz : (k + 1) * gsz],
                                xs[:, s + k : s + k + 128],
                                wds[og][:, k * gsz : (k + 1) * gsz],
                            )
                        nc.scalar.copy(
                            S[:, :, j, :],
                            P.rearrange("p (k q) -> p k q", k=_K),
                        )
                    k0, k1, k2 = (S[:, i, :, :] for i in range(_K))
                    tx = opool.tile([128, _J * gsz], f16, tag="tx", bufs=3)
                    tn = opool.tile([128, _J * gsz], f16, tag="tn", bufs=3)
                    nc.vector.tensor_tensor(tx[:], k0, k1, op=mx)
                    nc.vector.tensor_tensor(tn[:], k0, k1, op=mn)
                    nc.vector.tensor_tensor(tx[:], tx[:], k2, op=mx)
                    nc.vector.tensor_tensor(tn[:], tn[:], k2, op=mn)
                    # c-tree with o innermost: [p, j, c, oi], halve c to 1
                    oslc = slice(og_start * _OG, (og_start + _J) * _OG)
                    for side, src, dst, op in (
                        ("x", tx, ymax, mx),
                        ("n", tn, ymin, mn),
                    ):
                        cur = src.rearrange("p (j c o) -> p j c o", c=_C, o=_OG)
                        w = _C
                        while w > 2:
                            h = w // 2
                            nxt = opool.tile([128, _J, h, _OG], f16, tag=f"t{side}{h}")
                            nc.vector.tensor_tensor(
                                nxt[:], cur[:, :, 0:h, :], cur[:, :, h:w, :], op=op
                            )
                            cur = nxt
                            w = h
                        # last level writes straight into the ymax/ymin slice
                        nc.vector.tensor_tensor(
                            dst[:, oslc].rearrange("p (j o) -> p j o", o=_OG),
                            cur[:, :, 0, :],
                            cur[:, :, 1, :],
                            op=op,
                        )
                    og_start += _J
                ysum = opool.tile([128, _O], f32, tag="ysum")
                nc.vector.tensor_tensor(ysum[:], ymax[:], ymin[:], op=mybir.AluOpType.add)
                nc.sync.dma_start(yt_d[s : s + 128, :], ysum[:])

    nc.compile()
    return nc


def _get_module():
    if "nc" not in _cache:
        _cache["nc"] = _build_module()
    return _cache["nc"]


def _pack_weights(weight):
    # wd[og, c', k*512 + c*8 + oi] = (c'==c) * weight[og*8+oi, c, k]
    wq = weight.reshape(_O // _OG, _OG, _C, _K)  # [og, oi, c, k]
    wd = np.zeros((_O // _OG, _C, _K, _C, _OG), dtype=np.float32)
    ci = np.arange(_C)
    # wd[og, c, k, c, oi] = wq[og, oi, c, k]
    wd[:, ci, :, ci, :] = wq.transpose(2, 0, 3, 1)  # [c, og, k, oi]
    return np.ascontiguousarray(wd.reshape(_O // _OG, _C, _K * _OG * _C))


def kernel(x, weight, bias, stride):
    from concourse import bass_utils

    x = np.asarray(x, dtype=np.float32)
    weight = np.asarray(weight, dtype=np.float32)
    bias = np.asarray(bias, dtype=np.float32)
    assert int(stride) == 1
    assert x.shape == (_B, _C, _L) and weight.shape == (_O, _C, _K)

    nc = _get_module()

    wd = _pack_weights(weight).astype(np.float16)
    xp = np.zeros((_B, _C, _LPAD), dtype=np.float16)
    xp[:, :, :_L] = x

    in_maps = [{"x": xp[b], "wd": wd} for b in range(_B)]
    res = bass_utils.run_bass_kernel_spmd(nc, in_maps, core_ids=list(range(_B)))
    _cache["last_results"] = res

    y = np.empty((_B, _O, _LOUT), dtype=np.float32)
    for b in range(_B):
        y[b] = res.results[b]["yt"][:_LOUT, :].T
    y += bias[None, :, None]
    return y


# revision 5
# speedup vs baseline: 1.0103x; 1.0103x over previous
"""MAMConv1d Trainium2 kernel.

Y[b,o,l] = max_{c,k}(W[o,c,k] * x[b,c,l+k]) + min_{c,k}(...) + bias[o]
B=8, C=64, L=1024, O=64, K=3, stride=1, Lout=1022.

Strategy (8 NeuronCores, data-parallel over batch B). Per core b:
- Products are formed on the TensorEngine via block-diagonal matmuls:
  P[l, k*512 + c*8 + oi] = x[c, s+k+l] * W[o,c,k] with lhsT = x-window
  [64c, 128l] stationary and rhs = diag-expanded fp16 weights [64c, 512]
  (8 output channels per matmul). ScalarE casts each PSUM block to fp16;
  the VectorEngine runs the k-combine and a fully packed-2x c-halving
  tree (o innermost, so every level down to width 1 is a 2x
  tensor_tensor; no 1x tensor_reduce).
- The DVE is the saturated engine (~132us busy of ~140us): it must
  execute every max/min op. Measured constraints that pin this design:
  GpSimd rejects compare ALU ops (walrus ISA) and, worse, any running
  Pool-engine op stalls concurrent DVE ops even on disjoint tiles, so
  nothing (ysum included) runs there. fp16 PSUM matmul output is
  TRN3-only, so the ACT cast (~98us, hidden under DVE) is mandatory.
  tensor_reduce has no DVE perf modes (1x), hence the TT tree.
- bias is folded in on the host after the gather; output is written
  l-major [1024, 64] per core; host transposes/gathers.
"""

import numpy as np

_B, _C, _L = 8, 64, 1024
_O, _K = 64, 3
_LOUT = (_L - _K) + 1  # 1022
_LPAD = _L + 8
_OG = 8  # o-channels per matmul / reduce group
_NT = 8  # l-tiles of 128

_cache = {}

_MM_DTYPE = "float16"


def _build_module():
    import concourse.bacc as bacc
    import concourse.bass as bass
    import concourse.mybir as mybir
    import concourse.tile as tile

    f32 = mybir.dt.float32
    mmdt = getattr(mybir.dt, _MM_DTYPE)
    nc = bacc.Bacc("TRN2", target_bir_lowering=False, debug=False)

    x_d = nc.dram_tensor("x", [_C, _LPAD], mmdt, kind="ExternalInput")
    wd_d = nc.dram_tensor("wd", [_O // _OG, _C, _K * _OG * _C], mmdt, kind="ExternalInput")
    yt_d = nc.dram_tensor("yt", [_NT * 128, _O], f32, kind="ExternalOutput")

    n_og = _O // _OG  # 8 groups of 8 output channels
    gcols = _K * _OG * _C  # 1536 product columns per group

    with tile.TileContext(nc) as tc:
        with (
            tc.tile_pool(name="const", bufs=1) as cpool,
            tc.tile_pool(name="psum", bufs=2, space=bass.MemorySpace.PSUM) as ppool,
            tc.tile_pool(name="outp", bufs=3) as opool,
        ):
            gsz0 = _OG * _C  # 512
            xs = cpool.tile([_C, _LPAD], mmdt)
            wds = [cpool.tile([_C, gcols], mmdt, name=f"wds{og}") for og in range(n_og)]
            # Startup: first matmul needs xs[:, 0:130] + wds[0] k0; spread the
            # three wds[0] k-slabs across three DGE queues so they land in
            # parallel instead of serializing at ~600ns each.
            nc.scalar.dma_start(xs[:, 0:132], x_d[:, 0:132])
            nc.sync.dma_start(wds[0][:, 0:256], wd_d[0][:, 0:256])
            nc.sync.dma_start(wds[0][:, 256:512], wd_d[0][:, 256:512])
            nc.scalar.dma_start(wds[0][:, 512:1024], wd_d[0][:, 512:1024])
            nc.sync.dma_start(wds[0][:, 1024:1536], wd_d[0][:, 1024:1536])
            for xi, (a, b) in enumerate([(132, 432), (432, 732), (732, _LPAD)]):
                nc.scalar.dma_start(xs[:, a:b], x_d[:, a:b])
            for og in range(1, n_og):
                for k in range(_K):
                    cs = slice(k * gsz0, (k + 1) * gsz0)
                    nc.sync.dma_start(wds[og][:, cs], wd_d[og][:, cs])

            f16 = mybir.dt.float16
            gsz = _OG * _C  # 512 columns per k-plane

            mx, mn = mybir.AluOpType.max, mybir.AluOpType.min

            for t in range(_NT):
                s = 128 * t
                ymax = opool.tile([128, _O], f16, tag="ymax")
                ymin = opool.tile([128, _O], f16, tag="ymin")
                # graduated chunks fill the DVE pipeline early
                sched = {0: (1, 1, 2, 4), 1: (4, 4), _NT - 1: (4, 4)}.get(t, (8,))
                og_start = 0
                for _J in sched:
                    Sf = opool.tile([128, _K, 8, gsz], f16, tag="S", bufs=3)
                    S = Sf[:, :, :_J, :]
                    for j in range(_J):
                        og = og_start + j
                        P = ppool.tile([128, gcols], f32, tag="P")
                        for k in range(_K):
                            nc.tensor.matmul(
                                P[:, k * gsz : (k + 1) * gsz],
                                xs[:, s + k : s + k + 128],
                                wds[og][:, k * gsz : (k + 1) * gsz],
                            )
                        if t == 0 and _J == 1:
                            # startup: split the cast so the k-combine can
                            # start right after the k0/k1 matmuls land
                            nc.scalar.copy(
                                S[:, 0:2, j, :],
                                P[:, 0 : 2 * gsz].rearrange("p (k q) -> p k q", k=2),
                            )
                            nc.scalar.copy(S[:, 2, j, :], P[:, 2 * gsz : 3 * gsz])
                        else:
                            nc.scalar.copy(
                                S[:, :, j, :],
                                P.rearrange("p (k q) -> p k q", k=_K),
                            )
                    k0, k1, k2 = (S[:, i, :, :] for i in range(_K))
                    tx = opool.tile([128, _J * gsz], f16, tag="tx", bufs=3)
                    tn = opool.tile([128, _J * gsz], f16, tag="tn", bufs=3)
                    nc.vector.tensor_tensor(tx[:], k0, k1, op=mx)
                    nc.vector.tensor_tensor(tn[:], k0, k1, op=mn)
                    nc.vector.tensor_tensor(tx[:], tx[:], k2, op=mx)
                    nc.vector.tensor_tensor(tn[:], tn[:], k2, op=mn)
                    # c-tree with o innermost: [p, j, c, oi], halve c to 1
                    oslc = slice(og_start * _OG, (og_start + _J) * _OG)
                    for side, src, dst, op in (
                        ("x", tx, ymax, mx),
                        ("n", tn, ymin, mn),
                    ):
                        cur = src.rearrange("p (j c o) -> p j c o", c=_C, o=_OG)
                        w = _C
                        while w > 2:
                            h = w // 2
                            nxt = opool.tile([128, _J, h, _OG], f16, tag=f"t{side}{h}")
                            nc.vector.tensor_tensor(
                                nxt[:], cur[:, :, 0:h, :], cur[:, :, h:w, :], op=op
                            )
                            cur = nxt
                            w = h
                        # last level writes straight into the ymax/ymin slice
                        nc.vector.tensor_tensor(
                            dst[:, oslc].rearrange("p (j o) -> p j o", o=_OG),
                            cur[:, :, 0, :],
                            cur[:, :, 1, :],
                            op=op,
                        )
                    og_start += _J
                ysum = opool.tile([128, _O], f32, tag="ysum")
                nc.vector.tensor_tensor(ysum[:], ymax[:], ymin[:], op=mybir.AluOpType.add)
                nc.sync.dma_start(yt_d[s : s + 128, :], ysum[:])

    nc.compile()
    return nc


def _get_module():
    if "nc" not in _cache:
        _cache["nc"] = _build_module()
    return _cache["nc"]


def _pack_weights(weight):
    # wd[og, c', k*512 + c*8 + oi] = (c'==c) * weight[og*8+oi, c, k]
    wq = weight.reshape(_O // _OG, _OG, _C, _K)  # [og, oi, c, k]
    wd = np.zeros((_O // _OG, _C, _K, _C, _OG), dtype=np.float32)
    ci = np.arange(_C)
    # wd[og, c, k, c, oi] = wq[og, oi, c, k]
    wd[:, ci, :, ci, :] = wq.transpose(2, 0, 3, 1)  # [c, og, k, oi]
    return np.ascontiguousarray(wd.reshape(_O // _OG, _C, _K * _OG * _C))


def kernel(x, weight, bias, stride):
    from concourse import bass_utils

    x = np.asarray(x, dtype=np.float32)
    weight = np.asarray(weight, dtype=np.float32)
    bias = np.asarray(bias, dtype=np.float32)
    assert int(stride) == 1
    assert x.shape == (_B, _C, _L) and weight.shape == (_O, _C, _K)

    nc = _get_module()

    wd = _pack_weights(weight).astype(np.float16)
    xp = np.zeros((_B, _C, _LPAD), dtype=np.float16)
    xp[:, :, :_L] = x

    in_maps = [{"x": xp[b], "wd": wd} for b in range(_B)]
    res = bass_utils.run_bass_kernel_spmd(nc, in_maps, core_ids=list(range(_B)))
    _cache["last_results"] = res

    y = np.empty((_B, _O, _LOUT), dtype=np.float32)
    for b in range(_B):
        y[b] = res.results[b]["yt"][:_LOUT, :].T
    y += bias[None, :, None]
    return y
